# revision 10
# baseline (speedup 1.0000x reference)
"""BatchTreeEncoder Trainium2 kernel (8-core SPMD, data-parallel over batch).

Algorithm (mathematically identical to the reference, validated in numpy):
  reference: c = emb[tokens] @ W_c.T + b_c  -> bottom-up heap tree-sum -> max over nodes.

  Restructured:
    pass 1 (per-core vocab shard):  table = emb @ W_c.T        [30000, 128] f32
    pass 2 (per-core 32 trees):
      - dma_gather rows of `table` in a level-major slot order (nodes land on
        partitions, 128-node blocks on the free axis)
      - tree-sum levels computed as accumulating fp32r matmuls with constant
        0/1 pairing matrices (exact: multiplying by 1.0 is lossless)
      - bias handled as v[n] = h[n] - b_c: every internal level adds a uniform
        +2*b_c rank-1 term; the packed top block gets per-node weights w7;
        leaves need no bias at all.  Final: out = max_n v[n] + b_c.
      - max over nodes: elementwise max across the 8 blocks (DVE), then a
        TensorE transpose per tree + innermost reduce_max.
"""

import numpy as np
from contextlib import ExitStack

import concourse.bacc as bacc
import concourse.mybir as mybir
from concourse.bass_utils import run_bass_kernel_spmd
from concourse.library_config import mlp

F32 = mybir.dt.float32
F32R = mybir.dt.float32r
I16 = mybir.dt.int16
AF = mybir.ActivationFunctionType

VOCAB, EMB, ENC, BS, DEPTH = 30000, 128, 128, 256, 10
NNODE = 2 ** DEPTH - 1          # 1023
NCORES = 8
TPC = BS // NCORES              # 32 trees per core
G = 4                           # trees per group
NGROUP = TPC // G               # 8 groups per core
VSHARD = VOCAB // NCORES        # 3750
CHUNK = 480
NCHUNK = 8
VSHARD_PAD = CHUNK * NCHUNK     # 3840


# ----------------------------------------------------------------------------
# host-side constants
# ----------------------------------------------------------------------------

def _build_perm():
    """slot (0..1023) -> heap node index; slot 1023 is the pad."""
    perm = np.zeros(1024, dtype=np.int64)
    perm[0:512] = np.arange(511, 1023)      # blocks 0..3: leaves (level 9)
    perm[512:768] = np.arange(255, 511)     # blocks 4,5: level 8
    perm[768:896] = np.arange(127, 255)     # block 6: level 7
    p = 896                                 # block 7: levels 6..0 packed
    for lvl in range(6, -1, -1):
        s, e = 2 ** lvl - 1, 2 ** (lvl + 1) - 1
        perm[p:p + (e - s)] = np.arange(s, e)
        p += e - s
    perm[1023] = 0                          # pad slot gathers node-0's token (unused)
    return perm


def _build_mats(perm):
    SA = np.zeros((128, 128), np.float32)
    SB = np.zeros((128, 128), np.float32)
    for j in range(64):
        SA[2 * j, j] = 1.0
        SA[2 * j + 1, j] = 1.0
        SB[2 * j, j + 64] = 1.0
        SB[2 * j + 1, j + 64] = 1.0
    I = np.eye(128, dtype=np.float32)

    n7 = perm[896:1024]

    def is_anc(a, d):
        while True:
            if d == a:
                return True
            if d == 0:
                return False
            d = (d - 1) // 2

    A7 = np.zeros((128, 128), np.float32)
    B7 = np.zeros((128, 128), np.float32)
    w7 = np.zeros((128,), np.float32)
    for iloc in range(127):
        i = int(n7[iloc])
        for jloc in range(127):
            if is_anc(i, int(n7[jloc])):
                A7[jloc, iloc] = 1.0
        for k in range(128):
            if is_anc(i, 127 + k):
                B7[k, iloc] = 1.0
        lvl = int(np.floor(np.log2(i + 1)))
        w7[iloc] = 2.0 ** (8 - lvl) - 2.0
    # pad slot duplicates slot 0 so it can never incorrectly win the max
    A7[:, 127] = A7[:, 0]
    B7[:, 127] = B7[:, 0]
    w7[127] = w7[0]
    w2 = np.full((128,), 2.0, np.float32)
    return SA, SB, I, A7, B7, w2, w7


def _host_constants(W_c, b_c):
    perm = _build_perm()
    SA, SB, I, A7, B7, w2, w7 = _build_mats(perm)
    ident = np.eye(128, dtype=np.float32)
    mats = np.zeros((128, 7, 128), np.float32)
    for j, m in enumerate([SA, SB, I, A7, B7, ident]):
        mats[:, j, :] = m
    mats[:, 6, 0] = b_c
    brow = np.zeros((1, 768), np.float32)
    brow[0, 0:128] = w2
    brow[0, 128:256] = w7
    brow[0, 256:768] = np.tile(b_c, G)
    return perm, mats, brow


def _host_idx(tokens_core, perm):
    """tokens_core [32, 1023] -> idxw [128, 8, 4, 64] int16 (one 1024-idx gather
    per tree, wrapped in 16 partitions, replicated to 128)."""
    tok_pad = np.concatenate(
        [tokens_core, tokens_core[:, 0:1]], axis=1)       # [32, 1024]; pad col unused
    slots = tok_pad[:, perm]                              # [32, 1024], slot = B*128+p
    idxw = np.zeros((NGROUP, G, 16, 64), np.int16)
    for g in range(NGROUP):
        for t in range(G):
            idxw[g, t] = slots[g * G + t].reshape(64, 16).T
    idxw = np.tile(idxw, (1, 1, 8, 1))                    # [8, 4, 128, 64]
    return np.ascontiguousarray(idxw.transpose(2, 0, 1, 3))  # [128, 8, 4, 64]


# ----------------------------------------------------------------------------
# pass 1: table = emb @ W_c.T, vocab-sharded; output transposed [128, VSHARD_PAD]
# ----------------------------------------------------------------------------

def build_tablegen():
    nc = bacc.Bacc("TRN2", target_bir_lowering=False, debug=False)
    embt = nc.dram_tensor("embt", [128, VSHARD_PAD], F32R, kind="ExternalInput")
    wct = nc.dram_tensor("wct", [128, 128], F32R, kind="ExternalInput")
    tsh = nc.dram_tensor("tsh", [128, VSHARD_PAD], F32R, kind="ExternalOutput")
    with ExitStack() as ctx:
        embt_s = ctx.enter_context(nc.sbuf_tensor("embt_s", [128, VSHARD_PAD], F32R))
        wct_s = ctx.enter_context(nc.sbuf_tensor("wct_s", [128, 128], F32R))
        out_s = ctx.enter_context(nc.sbuf_tensor("out_s", [128, VSHARD_PAD], F32R))
        ps = [ctx.enter_context(nc.psum_tensor(f"ps{i}", [128, CHUNK], F32))
              for i in range(2)]
        s_in = ctx.enter_context(nc.semaphore("s_in"))
        s_mm = ctx.enter_context(nc.semaphore("s_mm"))
        s_cp = ctx.enter_context(nc.semaphore("s_cp"))
        s_out = ctx.enter_context(nc.semaphore("s_out"))
        with nc.Block() as block:
            @block.sync
            def _(eng):
                eng.dma_start(out=embt_s[:, :], in_=embt[:, :]).then_inc(s_in, 16)
                eng.dma_start(out=wct_s[:, :], in_=wct[:, :]).then_inc(s_in, 16)
                eng.wait_ge(s_cp, NCHUNK)
                eng.dma_start(out=tsh[:, :], in_=out_s[:, :]).then_inc(s_out, 16)
                eng.wait_ge(s_out, 16)

            @block.tensor
            def _(eng):
                eng.wait_ge(s_in, 32)
                for k in range(NCHUNK):
                    if k >= 2:
                        eng.wait_ge(s_cp, k - 1)
                    eng.matmul(
                        ps[k % 2][:, :],
                        wct_s[:, :],
                        embt_s[:, k * CHUNK:(k + 1) * CHUNK],
                        start=True, stop=True,
                    ).then_inc(s_mm, 1)

            @block.scalar
            def _(eng):
                for k in range(NCHUNK):
                    eng.wait_ge(s_mm, k + 1)
                    eng.activation(
                        out_s[:, k * CHUNK:(k + 1) * CHUNK], ps[k % 2][:, :],
                        AF.Copy,
                    ).then_inc(s_cp, 1)
    nc.compile()
    return nc


# ----------------------------------------------------------------------------
# pass 2: gather + tree-sum matmuls + max
# ----------------------------------------------------------------------------

def build_treeenc():
    nc = bacc.Bacc("TRN2", target_bir_lowering=False, debug=False)
    table = nc.dram_tensor("table", [VOCAB, ENC], F32R, kind="ExternalInput")
    idxw = nc.dram_tensor("idxw", [128, NGROUP, 4, 64], I16, kind="ExternalInput")
    matsd = nc.dram_tensor("mats", [128, 7, 128], F32R, kind="ExternalInput")
    browd = nc.dram_tensor("brow", [1, 768], F32R, kind="ExternalInput")
    m_out = nc.dram_tensor("m_out", [128, TPC], F32, kind="ExternalOutput")

    with ExitStack() as ctx:
        c_b = [ctx.enter_context(nc.sbuf_tensor(f"c{q}", [128, 4, 8, 128], F32R))
               for q in range(2)]
        h_b = [ctx.enter_context(nc.sbuf_tensor(f"h{q}", [128, 4, 512], F32R))
               for q in range(2)]
        mx_b = [ctx.enter_context(nc.sbuf_tensor(f"mx{q}", [128, 512], F32))
                for q in range(2)]
        tmpL = ctx.enter_context(nc.sbuf_tensor("tmpL", [128, 4, 2, 128], F32))
        tmp = [ctx.enter_context(nc.sbuf_tensor(f"tmp{i}", [128, 512], F32))
               for i in range(4)]
        M2_s = ctx.enter_context(nc.sbuf_tensor("M2_s", [128, TPC], F32))
        idx_s = ctx.enter_context(nc.sbuf_tensor("idx_s", [128, NGROUP, 4, 64], I16))
        mats_s = ctx.enter_context(nc.sbuf_tensor("mats_s", [128, 7, 128], F32R))
        brow_s = ctx.enter_context(nc.sbuf_tensor("brow_s", [1, 768], F32R))
        M_s = ctx.enter_context(nc.sbuf_tensor("M_s", [128, TPC], F32))

        # PSUM: 4 banks per parity: A, B, C, D.  A doubles as transpose target.
        pa = [ctx.enter_context(nc.psum_tensor(f"pa{q}", [128, 4, 128], F32))
              for q in range(2)]
        pb = [ctx.enter_context(nc.psum_tensor(f"pb{q}", [128, 4, 128], F32))
              for q in range(2)]
        pc = [ctx.enter_context(nc.psum_tensor(f"pc{q}", [128, 4, 128], F32))
              for q in range(2)]
        pd = [ctx.enter_context(nc.psum_tensor(f"pd{q}", [128, 4, 128], F32))
              for q in range(2)]

        s_init = ctx.enter_context(nc.semaphore("s_init"))
        s_gat = [ctx.enter_context(nc.semaphore(f"s_gat{q}")) for q in range(2)]
        s_mm = [ctx.enter_context(nc.semaphore(f"s_mm{x}")) for x in "ABCD"]
        s_act = [ctx.enter_context(nc.semaphore(f"s_act{x}")) for x in "ABCD"]
        s_leaf = ctx.enter_context(nc.semaphore("s_leaf"))
        s_mx = ctx.enter_context(nc.semaphore("s_mx"))
        s_tr = ctx.enter_context(nc.semaphore("s_tr"))
        s_red = ctx.enter_context(nc.semaphore("s_red"))
        s_fin = ctx.enter_context(nc.semaphore("s_fin"))
        s_out = ctx.enter_context(nc.semaphore("s_out"))

        SAa = mats_s[:, 0, :]
        SBa = mats_s[:, 1, :]
        Ia = mats_s[:, 2, :]
        A7a = mats_s[:, 3, :]
        B7a = mats_s[:, 4, :]
        identa = mats_s[:, 5, :].bitcast(F32)
        bcol = mats_s[:, 6, 0:1].bitcast(F32)
        w2a = brow_s[:, 0:128]
        w7a = brow_s[:, 128:256]
        bcra = brow_s[:, 256:768]

        def cblk(q, B):
            return c_b[q][:, :, B, :]

        with nc.Block() as block:
            @block.sync
            def _(eng):
                eng.dma_start(out=idx_s[:, :, :, :], in_=idxw[:, :, :, :]).then_inc(s_init, 16)
                eng.dma_start(out=mats_s[:, :, :], in_=matsd[:, :, :]).then_inc(s_init, 16)
                eng.dma_start(out=brow_s[:, :], in_=browd[:, :]).then_inc(s_init, 16)
                eng.wait_ge(s_fin, 1)
                eng.dma_start(out=m_out[:, :], in_=M2_s[:, :]).then_inc(s_out, 16)
                eng.wait_ge(s_out, 16)

            @block.gpsimd
            def _(eng):
                eng.load_library(mlp)
                eng.wait_ge(s_init, 48)
                for g in range(NGROUP):
                    q = g % 2
                    if g >= 2:
                        eng.wait_ge(s_mm[3], g - 1)   # PE done with c[q] (bankD reads c7)
                        eng.wait_ge(s_leaf, g - 1)    # DVE done with leaf blocks of c[q]
                    for t in range(G):
                        eng.dma_gather(
                            c_b[q][:, t, :, :], table[:, :], idx_s[:, g, t, :],
                            1024, 1024, ENC,
                        ).then_inc(s_gat[q], 16)

            @block.tensor
            def _(eng):
                eng.wait_ge(s_init, 48)
                for g in range(NGROUP):
                    q = g % 2
                    eng.wait_ge(s_gat[q], 64 * (g // 2 + 1))
                    # bank A: level-8 parents (tree-block 4)
                    if g >= 2:
                        eng.wait_ge(s_red, g - 1)     # pa[q] free (DVE reduce done)
                    eng.matmul(pa[q][:, :, :], SAa, cblk(q, 0), start=True, stop=False)
                    eng.matmul(pa[q][:, :, :], SBa, cblk(q, 1), start=False, stop=False)
                    eng.matmul(pa[q][:, :, :], Ia, cblk(q, 4), start=False, stop=False)
                    eng.matmul(pa[q][:, :, :], w2a, bcra, start=False, stop=True
                               ).then_inc(s_mm[0], 1)
                    # bank B: level-8 parents (tree-block 5)
                    if g >= 2:
                        eng.wait_ge(s_act[1], g - 1)  # pb[q] free
                    eng.matmul(pb[q][:, :, :], SAa, cblk(q, 2), start=True, stop=False)
                    eng.matmul(pb[q][:, :, :], SBa, cblk(q, 3), start=False, stop=False)
                    eng.matmul(pb[q][:, :, :], Ia, cblk(q, 5), start=False, stop=False)
                    eng.matmul(pb[q][:, :, :], w2a, bcra, start=False, stop=True
                               ).then_inc(s_mm[1], 1)
                    # bank C: level-7 parents, children are h0, h1
                    eng.wait_ge(s_act[0], g + 1)
                    eng.wait_ge(s_act[1], g + 1)
                    if g >= 2:
                        eng.wait_ge(s_act[2], g - 1)  # pc[q] free
                    eng.matmul(pc[q][:, :, :], SAa, h_b[q][:, 0, :],
                               start=True, stop=False)
                    eng.matmul(pc[q][:, :, :], SBa, h_b[q][:, 1, :],
                               start=False, stop=False)
                    eng.matmul(pc[q][:, :, :], Ia, cblk(q, 6), start=False, stop=False)
                    eng.matmul(pc[q][:, :, :], w2a, bcra, start=False, stop=True
                               ).then_inc(s_mm[2], 1)
                    # bank D: packed top block (levels 6..0)
                    eng.wait_ge(s_act[2], g + 1)
                    if g >= 2:
                        eng.wait_ge(s_act[3], g - 1)  # pd[q] free
                    eng.matmul(pd[q][:, :, :], A7a, cblk(q, 7), start=True, stop=False)
                    eng.matmul(pd[q][:, :, :], B7a, h_b[q][:, 2, :],
                               start=False, stop=False)
                    eng.matmul(pd[q][:, :, :], w7a, bcra, start=False, stop=True
                               ).then_inc(s_mm[3], 1)
                    # transposes of the block-max into pa[q] (pa already copied out)
                    eng.wait_ge(s_mx, g + 1)
                    for t in range(G):
                        ins = eng.transpose(pa[q][:, t, :],
                                            mx_b[q][:, t * 128:(t + 1) * 128],
                                            identa)
                    ins.then_inc(s_tr, 1)

            @block.scalar
            def _(eng):
                for g in range(NGROUP):
                    q = g % 2
                    if g >= 2:
                        eng.wait_ge(s_mx, g - 1)      # h[q] free (DVE maxes done)
                    for x in range(4):
                        eng.wait_ge(s_mm[x], g + 1)
                        src = [pa, pb, pc, pd][x]
                        eng.activation(h_b[q][:, x, :], src[q][:, :, :], AF.Copy
                                       ).then_inc(s_act[x], 1)

            @block.vector
            def _(eng):
                for g in range(NGROUP):
                    q = g % 2
                    eng.wait_ge(s_gat[q], 64 * (g // 2 + 1))
                    # leaf max over c blocks 0..3 (per tree)
                    eng.tensor_max(tmpL[:, :, :, :], c_b[q][:, :, 0:2, :].bitcast(F32),
                                   c_b[q][:, :, 2:4, :].bitcast(F32)
                                   ).then_inc(s_leaf, 1)
                    # internal max over h blocks
                    eng.wait_ge(s_act[3], g + 1)
                    eng.tensor_max(tmp[0][:, :], h_b[q][:, 0, :].bitcast(F32),
                                   h_b[q][:, 1, :].bitcast(F32))
                    eng.tensor_max(tmp[1][:, :], h_b[q][:, 2, :].bitcast(F32),
                                   h_b[q][:, 3, :].bitcast(F32))
                    eng.drain()
                    eng.tensor_max(tmp[2][:, :], tmpL[:, :, 0, :], tmpL[:, :, 1, :])
                    eng.tensor_max(tmp[3][:, :], tmp[0][:, :], tmp[1][:, :])
                    eng.drain()
                    if g >= 2:
                        eng.wait_ge(s_tr, g - 1)      # mx[q] free (PE transposes done)
                    eng.tensor_max(mx_b[q][:, :], tmp[2][:, :], tmp[3][:, :]
                                   ).then_inc(s_mx, 1)
                    # per-tree reduction of transposed maxes
                    eng.wait_ge(s_tr, g + 1)
                    eng.reduce_max(M_s[:, G * g:G * (g + 1)], pa[q][:, :, :],
                                   axis=mybir.AxisListType.X).then_inc(s_red, 1)
                eng.drain()
                eng.tensor_scalar_add(M2_s[:, :], M_s[:, :], bcol).then_inc(s_fin, 1)
    nc.compile()
    return nc


# ----------------------------------------------------------------------------
# host orchestration
# ----------------------------------------------------------------------------

_CACHE = {}


def _get_programs():
    if "nc1" not in _CACHE:
        _CACHE["nc1"] = build_tablegen()
        _CACHE["nc2"] = build_treeenc()
    return _CACHE["nc1"], _CACHE["nc2"]


def _pass1_inputs(emb, W_c):
    embT = np.zeros((128, NCORES, VSHARD_PAD), np.float32)
    embT[:, :, :VSHARD] = np.ascontiguousarray(emb.T).reshape(128, NCORES, VSHARD)
    wct = np.ascontiguousarray(W_c.T)
    return [{"embt": np.ascontiguousarray(embT[:, i, :]), "wct": wct}
            for i in range(NCORES)]


def _assemble_table(results):
    shards = [results[i]["tsh"][:, :VSHARD] for i in range(NCORES)]
    return np.ascontiguousarray(np.concatenate(shards, axis=1).T)  # [30000, 128]


def _pass2_inputs(tokens, table, W_c, b_c):
    perm, mats, brow = _host_constants(W_c, b_c)
    in2 = []
    for i in range(NCORES):
        tc = tokens[i * TPC:(i + 1) * TPC]
        in2.append({
            "table": table,
            "idxw": _host_idx(tc, perm),
            "mats": mats,
            "brow": brow,
        })
    return in2


def _assemble_out(results):
    out = np.zeros((BS, ENC), np.float32)
    for i in range(NCORES):
        out[i * TPC:(i + 1) * TPC] = results[i]["m_out"].T
    return out


def kernel(tokens, emb, W_c, b_c, bs=None):
    tokens = np.asarray(tokens).astype(np.int64)
    emb = np.asarray(emb, np.float32)
    W_c = np.asarray(W_c, np.float32)
    b_c = np.asarray(b_c, np.float32)
    assert tokens.shape == (BS, NNODE) and emb.shape == (VOCAB, EMB)

    nc1, nc2 = _get_programs()
    core_ids = list(range(NCORES))

    r1 = run_bass_kernel_spmd(nc1, _pass1_inputs(emb, W_c), core_ids)
    table = _assemble_table(r1.results)
    r2 = run_bass_kernel_spmd(nc2, _pass2_inputs(tokens, table, W_c, b_c),
                              core_ids)
    return _assemble_out(r2.results)


# revision 12
# speedup vs baseline: 1.0095x; 1.0095x over previous
"""BatchTreeEncoder Trainium2 kernel (8-core SPMD, data-parallel over batch).

Algorithm (mathematically identical to the reference, validated in numpy):
  reference: c = emb[tokens] @ W_c.T + b_c  -> bottom-up heap tree-sum -> max over nodes.

  Restructured:
    pass 1 (per-core vocab shard):  table = emb @ W_c.T        [30000, 128] f32
    pass 2 (per-core 32 trees):
      - dma_gather rows of `table` in a level-major slot order (nodes land on
        partitions, 128-node blocks on the free axis)
      - tree-sum levels computed as accumulating fp32r matmuls with constant
        0/1 pairing matrices (exact: multiplying by 1.0 is lossless)
      - bias handled as v[n] = h[n] - b_c: every internal level adds a uniform
        +2*b_c rank-1 term; the packed top block gets per-node weights w7;
        leaves need no bias at all.  Final: out = max_n v[n] + b_c.
      - max over nodes: elementwise max across the 8 blocks (DVE), then a
        TensorE transpose per tree + innermost reduce_max.
"""

import numpy as np
from contextlib import ExitStack

import concourse.bacc as bacc
import concourse.mybir as mybir
from concourse.bass_utils import run_bass_kernel_spmd
from concourse.library_config import mlp

F32 = mybir.dt.float32
F32R = mybir.dt.float32r
I16 = mybir.dt.int16
AF = mybir.ActivationFunctionType

VOCAB, EMB, ENC, BS, DEPTH = 30000, 128, 128, 256, 10
NNODE = 2 ** DEPTH - 1          # 1023
NCORES = 8
TPC = BS // NCORES              # 32 trees per core
G = 4                           # trees per group
NGROUP = TPC // G               # 8 groups per core
VSHARD = VOCAB // NCORES        # 3750
CHUNK = 480
NCHUNK = 8
VSHARD_PAD = CHUNK * NCHUNK     # 3840


# ----------------------------------------------------------------------------
# host-side constants
# ----------------------------------------------------------------------------

def _build_perm():
    """slot (0..1023) -> heap node index; slot 1023 is the pad."""
    perm = np.zeros(1024, dtype=np.int64)
    perm[0:512] = np.arange(511, 1023)      # blocks 0..3: leaves (level 9)
    perm[512:768] = np.arange(255, 511)     # blocks 4,5: level 8
    perm[768:896] = np.arange(127, 255)     # block 6: level 7
    p = 896                                 # block 7: levels 6..0 packed
    for lvl in range(6, -1, -1):
        s, e = 2 ** lvl - 1, 2 ** (lvl + 1) - 1
        perm[p:p + (e - s)] = np.arange(s, e)
        p += e - s
    perm[1023] = 0                          # pad slot gathers node-0's token (unused)
    return perm


def _build_mats(perm):
    SA = np.zeros((128, 128), np.float32)
    SB = np.zeros((128, 128), np.float32)
    for j in range(64):
        SA[2 * j, j] = 1.0
        SA[2 * j + 1, j] = 1.0
        SB[2 * j, j + 64] = 1.0
        SB[2 * j + 1, j + 64] = 1.0
    I = np.eye(128, dtype=np.float32)

    n7 = perm[896:1024]

    def is_anc(a, d):
        while True:
            if d == a:
                return True
            if d == 0:
                return False
            d = (d - 1) // 2

    A7 = np.zeros((128, 128), np.float32)
    B7 = np.zeros((128, 128), np.float32)
    w7 = np.zeros((128,), np.float32)
    for iloc in range(127):
        i = int(n7[iloc])
        for jloc in range(127):
            if is_anc(i, int(n7[jloc])):
                A7[jloc, iloc] = 1.0
        for k in range(128):
            if is_anc(i, 127 + k):
                B7[k, iloc] = 1.0
        lvl = int(np.floor(np.log2(i + 1)))
        w7[iloc] = 2.0 ** (8 - lvl) - 2.0
    # pad slot duplicates slot 0 so it can never incorrectly win the max
    A7[:, 127] = A7[:, 0]
    B7[:, 127] = B7[:, 0]
    w7[127] = w7[0]
    w2 = np.full((128,), 2.0, np.float32)
    return SA, SB, I, A7, B7, w2, w7


def _host_constants(W_c, b_c):
    perm = _build_perm()
    SA, SB, I, A7, B7, w2, w7 = _build_mats(perm)
    ident = np.eye(128, dtype=np.float32)
    mats = np.zeros((128, 7, 128), np.float32)
    for j, m in enumerate([SA, SB, I, A7, B7, ident]):
        mats[:, j, :] = m
    mats[:, 6, 0] = b_c
    brow = np.zeros((1, 768), np.float32)
    brow[0, 0:128] = w2
    brow[0, 128:256] = w7
    brow[0, 256:768] = np.tile(b_c, G)
    return perm, mats, brow


def _host_idx(tokens_core, perm):
    """tokens_core [32, 1023] -> idxw [128, 8, 4, 64] int16 (one 1024-idx gather
    per tree, wrapped in 16 partitions, replicated to 128)."""
    tok_pad = np.concatenate(
        [tokens_core, tokens_core[:, 0:1]], axis=1)       # [32, 1024]; pad col unused
    slots = tok_pad[:, perm]                              # [32, 1024], slot = B*128+p
    idxw = np.zeros((NGROUP, G, 16, 64), np.int16)
    for g in range(NGROUP):
        for t in range(G):
            idxw[g, t] = slots[g * G + t].reshape(64, 16).T
    idxw = np.tile(idxw, (1, 1, 8, 1))                    # [8, 4, 128, 64]
    return np.ascontiguousarray(idxw.transpose(2, 0, 1, 3))  # [128, 8, 4, 64]


# ----------------------------------------------------------------------------
# pass 1: table = emb @ W_c.T, vocab-sharded; output transposed [128, VSHARD_PAD]
# ----------------------------------------------------------------------------

def build_tablegen():
    nc = bacc.Bacc("TRN2", target_bir_lowering=False, debug=False)
    embt = nc.dram_tensor("embt", [128, VSHARD_PAD], F32R, kind="ExternalInput")
    wct = nc.dram_tensor("wct", [128, 128], F32R, kind="ExternalInput")
    tsh = nc.dram_tensor("tsh", [128, VSHARD_PAD], F32R, kind="ExternalOutput")
    with ExitStack() as ctx:
        embt_s = ctx.enter_context(nc.sbuf_tensor("embt_s", [128, VSHARD_PAD], F32R))
        wct_s = ctx.enter_context(nc.sbuf_tensor("wct_s", [128, 128], F32R))
        out_s = ctx.enter_context(nc.sbuf_tensor("out_s", [128, VSHARD_PAD], F32R))
        ps = [ctx.enter_context(nc.psum_tensor(f"ps{i}", [128, CHUNK], F32))
              for i in range(2)]
        s_in = ctx.enter_context(nc.semaphore("s_in"))
        s_mm = ctx.enter_context(nc.semaphore("s_mm"))
        s_cp = ctx.enter_context(nc.semaphore("s_cp"))
        s_out = ctx.enter_context(nc.semaphore("s_out"))
        with nc.Block() as block:
            @block.sync
            def _(eng):
                eng.dma_start(out=embt_s[:, :], in_=embt[:, :]).then_inc(s_in, 16)
                eng.dma_start(out=wct_s[:, :], in_=wct[:, :]).then_inc(s_in, 16)
                eng.wait_ge(s_cp, NCHUNK)
                eng.dma_start(out=tsh[:, :], in_=out_s[:, :]).then_inc(s_out, 16)
                eng.wait_ge(s_out, 16)

            @block.tensor
            def _(eng):
                eng.wait_ge(s_in, 32)
                for k in range(NCHUNK):
                    if k >= 2:
                        eng.wait_ge(s_cp, k - 1)
                    eng.matmul(
                        ps[k % 2][:, :],
                        wct_s[:, :],
                        embt_s[:, k * CHUNK:(k + 1) * CHUNK],
                        start=True, stop=True,
                    ).then_inc(s_mm, 1)

            @block.scalar
            def _(eng):
                for k in range(NCHUNK):
                    eng.wait_ge(s_mm, k + 1)
                    eng.activation(
                        out_s[:, k * CHUNK:(k + 1) * CHUNK], ps[k % 2][:, :],
                        AF.Copy,
                    ).then_inc(s_cp, 1)
    nc.compile()
    return nc


# ----------------------------------------------------------------------------
# pass 2: gather + tree-sum matmuls + max
# ----------------------------------------------------------------------------

def build_treeenc():
    nc = bacc.Bacc("TRN2", target_bir_lowering=False, debug=False)
    table = nc.dram_tensor("table", [VOCAB, ENC], F32R, kind="ExternalInput")
    idxw = nc.dram_tensor("idxw", [128, NGROUP, 4, 64], I16, kind="ExternalInput")
    matsd = nc.dram_tensor("mats", [128, 7, 128], F32R, kind="ExternalInput")
    browd = nc.dram_tensor("brow", [1, 768], F32R, kind="ExternalInput")
    m_out = nc.dram_tensor("m_out", [128, TPC], F32, kind="ExternalOutput")

    with ExitStack() as ctx:
        c_b = [ctx.enter_context(nc.sbuf_tensor(f"c{q}", [128, 4, 8, 128], F32R))
               for q in range(2)]
        h_b = [ctx.enter_context(nc.sbuf_tensor(f"h{q}", [128, 4, 512], F32R))
               for q in range(2)]
        mx_b = [ctx.enter_context(nc.sbuf_tensor(f"mx{q}", [128, 512], F32))
                for q in range(2)]
        tmpL = ctx.enter_context(nc.sbuf_tensor("tmpL", [128, 4, 2, 128], F32))
        tmp = [ctx.enter_context(nc.sbuf_tensor(f"tmp{i}", [128, 512], F32))
               for i in range(4)]
        M2_s = ctx.enter_context(nc.sbuf_tensor("M2_s", [128, TPC], F32))
        idx_s = ctx.enter_context(nc.sbuf_tensor("idx_s", [128, NGROUP, 4, 64], I16))
        mats_s = ctx.enter_context(nc.sbuf_tensor("mats_s", [128, 7, 128], F32R))
        brow_s = ctx.enter_context(nc.sbuf_tensor("brow_s", [1, 768], F32R))
        M_s = ctx.enter_context(nc.sbuf_tensor("M_s", [128, TPC], F32))

        # PSUM: 4 banks per parity: A, B, C, D.  A doubles as transpose target.
        pa = [ctx.enter_context(nc.psum_tensor(f"pa{q}", [128, 4, 128], F32))
              for q in range(2)]
        pb = [ctx.enter_context(nc.psum_tensor(f"pb{q}", [128, 4, 128], F32))
              for q in range(2)]
        pc = [ctx.enter_context(nc.psum_tensor(f"pc{q}", [128, 4, 128], F32))
              for q in range(2)]
        pd = [ctx.enter_context(nc.psum_tensor(f"pd{q}", [128, 4, 128], F32))
              for q in range(2)]

        s_init = ctx.enter_context(nc.semaphore("s_init"))
        s_gat = [ctx.enter_context(nc.semaphore(f"s_gat{q}")) for q in range(2)]
        s_mm = [ctx.enter_context(nc.semaphore(f"s_mm{x}")) for x in "ABCD"]
        s_act = [ctx.enter_context(nc.semaphore(f"s_act{x}")) for x in "ABCD"]
        s_leaf = ctx.enter_context(nc.semaphore("s_leaf"))
        s_mx = ctx.enter_context(nc.semaphore("s_mx"))
        s_tr = ctx.enter_context(nc.semaphore("s_tr"))
        s_red = ctx.enter_context(nc.semaphore("s_red"))
        s_fin = ctx.enter_context(nc.semaphore("s_fin"))
        s_out = ctx.enter_context(nc.semaphore("s_out"))
        s_gfin = ctx.enter_context(nc.semaphore("s_gfin"))

        SAa = mats_s[:, 0, :]
        SBa = mats_s[:, 1, :]
        Ia = mats_s[:, 2, :]
        A7a = mats_s[:, 3, :]
        B7a = mats_s[:, 4, :]
        identa = mats_s[:, 5, :].bitcast(F32)
        bcol = mats_s[:, 6, 0:1].bitcast(F32)
        w2a = brow_s[:, 0:128]
        w7a = brow_s[:, 128:256]
        bcra = brow_s[:, 256:768]

        def cblk(q, B):
            return c_b[q][:, :, B, :]

        with nc.Block(no_gpsimd_drain=True) as block:
            @block.sync
            def _(eng):
                eng.dma_start(out=idx_s[:, :, :, :], in_=idxw[:, :, :, :]).then_inc(s_init, 16)
                eng.dma_start(out=mats_s[:, :, :], in_=matsd[:, :, :]).then_inc(s_init, 16)
                eng.dma_start(out=brow_s[:, :], in_=browd[:, :]).then_inc(s_init, 16)
                eng.wait_ge(s_fin, 1)
                eng.dma_start(out=m_out[:, :], in_=M2_s[:, :]).then_inc(s_out, 16)
                eng.wait_ge(s_out, 16)

            @block.gpsimd
            def _(eng):
                eng.load_library(mlp)
                eng.wait_ge(s_init, 48)
                for g in range(NGROUP):
                    q = g % 2
                    if g >= 2:
                        eng.wait_ge(s_mm[3], g - 1)   # PE done with c[q] (bankD reads c7)
                        eng.wait_ge(s_leaf, g - 1)    # DVE done with leaf blocks of c[q]
                    for t in range(G):
                        eng.dma_gather(
                            c_b[q][:, t, :, :], table[:, :], idx_s[:, g, t, :],
                            1024, 1024, ENC,
                        ).then_inc(s_gat[q], 16)

            @block.tensor
            def _(eng):
                eng.wait_ge(s_init, 48)
                for g in range(NGROUP):
                    q = g % 2
                    eng.wait_ge(s_gat[q], 64 * (g // 2 + 1))
                    # bank A: level-8 parents (tree-block 4)
                    if g >= 2:
                        eng.wait_ge(s_red, g - 1)     # pa[q] free (DVE reduce done)
                    eng.matmul(pa[q][:, :, :], SAa, cblk(q, 0), start=True, stop=False)
                    eng.matmul(pa[q][:, :, :], SBa, cblk(q, 1), start=False, stop=False)
                    eng.matmul(pa[q][:, :, :], Ia, cblk(q, 4), start=False, stop=False)
                    eng.matmul(pa[q][:, :, :], w2a, bcra, start=False, stop=True
                               ).then_inc(s_mm[0], 1)
                    # bank B: level-8 parents (tree-block 5)
                    if g >= 2:
                        eng.wait_ge(s_act[1], g - 1)  # pb[q] free
                    eng.matmul(pb[q][:, :, :], SAa, cblk(q, 2), start=True, stop=False)
                    eng.matmul(pb[q][:, :, :], SBa, cblk(q, 3), start=False, stop=False)
                    eng.matmul(pb[q][:, :, :], Ia, cblk(q, 5), start=False, stop=False)
                    eng.matmul(pb[q][:, :, :], w2a, bcra, start=False, stop=True
                               ).then_inc(s_mm[1], 1)
                    # bank C: level-7 parents, children are h0, h1
                    eng.wait_ge(s_act[0], g + 1)
                    eng.wait_ge(s_act[1], g + 1)
                    if g >= 2:
                        eng.wait_ge(s_act[2], g - 1)  # pc[q] free
                    eng.matmul(pc[q][:, :, :], SAa, h_b[q][:, 0, :],
                               start=True, stop=False)
                    eng.matmul(pc[q][:, :, :], SBa, h_b[q][:, 1, :],
                               start=False, stop=False)
                    eng.matmul(pc[q][:, :, :], Ia, cblk(q, 6), start=False, stop=False)
                    eng.matmul(pc[q][:, :, :], w2a, bcra, start=False, stop=True
                               ).then_inc(s_mm[2], 1)
                    # bank D: packed top block (levels 6..0)
                    eng.wait_ge(s_act[2], g + 1)
                    if g >= 2:
                        eng.wait_ge(s_act[3], g - 1)  # pd[q] free
                    eng.matmul(pd[q][:, :, :], A7a, cblk(q, 7), start=True, stop=False)
                    eng.matmul(pd[q][:, :, :], B7a, h_b[q][:, 2, :],
                               start=False, stop=False)
                    eng.matmul(pd[q][:, :, :], w7a, bcra, start=False, stop=True
                               ).then_inc(s_mm[3], 1)
                    # transposes of the block-max into pa[q] (pa already copied out)
                    eng.wait_ge(s_mx, g + 1)
                    for t in range(G):
                        ins = eng.transpose(pa[q][:, t, :],
                                            mx_b[q][:, t * 128:(t + 1) * 128],
                                            identa)
                    ins.then_inc(s_tr, 1)

            @block.scalar
            def _(eng):
                for g in range(NGROUP):
                    q = g % 2
                    if g >= 2:
                        eng.wait_ge(s_mx, g - 1)      # h[q] free (DVE maxes done)
                    for x in range(4):
                        eng.wait_ge(s_mm[x], g + 1)
                        src = [pa, pb, pc, pd][x]
                        eng.activation(h_b[q][:, x, :], src[q][:, :, :], AF.Copy
                                       ).then_inc(s_act[x], 1)

            @block.vector
            def _(eng):
                for g in range(NGROUP):
                    q = g % 2
                    eng.wait_ge(s_gat[q], 64 * (g // 2 + 1))
                    # leaf max over c blocks 0..3 (per tree)
                    eng.tensor_max(tmpL[:, :, :, :], c_b[q][:, :, 0:2, :].bitcast(F32),
                                   c_b[q][:, :, 2:4, :].bitcast(F32)
                                   ).then_inc(s_leaf, 1)
                    # internal max over h blocks
                    eng.wait_ge(s_act[3], g + 1)
                    eng.tensor_max(tmp[0][:, :], h_b[q][:, 0, :].bitcast(F32),
                                   h_b[q][:, 1, :].bitcast(F32))
                    eng.tensor_max(tmp[1][:, :], h_b[q][:, 2, :].bitcast(F32),
                                   h_b[q][:, 3, :].bitcast(F32))
                    eng.drain()
                    eng.tensor_max(tmp[2][:, :], tmpL[:, :, 0, :], tmpL[:, :, 1, :])
                    eng.tensor_max(tmp[3][:, :], tmp[0][:, :], tmp[1][:, :])
                    eng.drain()
                    if g >= 2:
                        eng.wait_ge(s_tr, g - 1)      # mx[q] free (PE transposes done)
                    eng.tensor_max(mx_b[q][:, :], tmp[2][:, :], tmp[3][:, :]
                                   ).then_inc(s_mx, 1)
                    # per-tree reduction of transposed maxes
                    eng.wait_ge(s_tr, g + 1)
                    eng.reduce_max(M_s[:, G * g:G * (g + 1)], pa[q][:, :, :],
                                   axis=mybir.AxisListType.X).then_inc(s_red, 1)
                eng.drain()
                eng.tensor_scalar_add(M2_s[:, :], M_s[:, :], bcol).then_inc(s_fin, 1)
    nc.compile()
    return nc


# ----------------------------------------------------------------------------
# host orchestration
# ----------------------------------------------------------------------------

_CACHE = {}


def _get_programs():
    if "nc1" not in _CACHE:
        _CACHE["nc1"] = build_tablegen()
        _CACHE["nc2"] = build_treeenc()
    return _CACHE["nc1"], _CACHE["nc2"]


def _pass1_inputs(emb, W_c):
    embT = np.zeros((128, NCORES, VSHARD_PAD), np.float32)
    embT[:, :, :VSHARD] = np.ascontiguousarray(emb.T).reshape(128, NCORES, VSHARD)
    wct = np.ascontiguousarray(W_c.T)
    return [{"embt": np.ascontiguousarray(embT[:, i, :]), "wct": wct}
            for i in range(NCORES)]


def _assemble_table(results):
    shards = [results[i]["tsh"][:, :VSHARD] for i in range(NCORES)]
    return np.ascontiguousarray(np.concatenate(shards, axis=1).T)  # [30000, 128]


def _pass2_inputs(tokens, table, W_c, b_c):
    perm, mats, brow = _host_constants(W_c, b_c)
    in2 = []
    for i in range(NCORES):
        tc = tokens[i * TPC:(i + 1) * TPC]
        in2.append({
            "table": table,
            "idxw": _host_idx(tc, perm),
            "mats": mats,
            "brow": brow,
        })
    return in2


def _assemble_out(results):
    out = np.zeros((BS, ENC), np.float32)
    for i in range(NCORES):
        out[i * TPC:(i + 1) * TPC] = results[i]["m_out"].T
    return out


def kernel(tokens, emb, W_c, b_c, bs=None):
    tokens = np.asarray(tokens).astype(np.int64)
    emb = np.asarray(emb, np.float32)
    W_c = np.asarray(W_c, np.float32)
    b_c = np.asarray(b_c, np.float32)
    assert tokens.shape == (BS, NNODE) and emb.shape == (VOCAB, EMB)

    nc1, nc2 = _get_programs()
    core_ids = list(range(NCORES))

    r1 = run_bass_kernel_spmd(nc1, _pass1_inputs(emb, W_c), core_ids)
    table = _assemble_table(r1.results)
    r2 = run_bass_kernel_spmd(nc2, _pass2_inputs(tokens, table, W_c, b_c),
                              core_ids)
    return _assemble_out(r2.results)


# revision 14
# speedup vs baseline: 1.7754x; 1.7587x over previous
"""BatchTreeEncoder Trainium2 kernel (8-core SPMD, data-parallel over batch).

Algorithm (mathematically identical to the reference, validated in numpy):
  reference: c = emb[tokens] @ W_c.T + b_c  -> bottom-up heap tree-sum -> max over nodes.

  Restructured:
    pass 1 (per-core vocab shard):  table = emb @ W_c.T        [30000, 128] f32
    pass 2 (per-core 32 trees):
      - dma_gather rows of `table` in a level-major slot order (nodes land on
        partitions, 128-node blocks on the free axis)
      - tree-sum levels computed as accumulating fp32r matmuls with constant
        0/1 pairing matrices (exact: multiplying by 1.0 is lossless)
      - bias handled as v[n] = h[n] - b_c: every internal level adds a uniform
        +2*b_c rank-1 term; the packed top block gets per-node weights w7;
        leaves need no bias at all.  Final: out = max_n v[n] + b_c.
      - max over nodes: elementwise max across the 8 blocks (DVE), then a
        TensorE transpose per tree + innermost reduce_max.
"""

import numpy as np
from contextlib import ExitStack

import concourse.bacc as bacc
import concourse.mybir as mybir
from concourse.bass_utils import run_bass_kernel_spmd
from concourse.library_config import mlp

F32 = mybir.dt.float32
F32R = mybir.dt.float32r
I16 = mybir.dt.int16
AF = mybir.ActivationFunctionType

VOCAB, EMB, ENC, BS, DEPTH = 30000, 128, 128, 256, 10
NNODE = 2 ** DEPTH - 1          # 1023
NCORES = 8
TPC = BS // NCORES              # 32 trees per core
G = 4                           # trees per group
NGROUP = TPC // G               # 8 groups per core
VSHARD = VOCAB // NCORES        # 3750
CHUNK = 480
NCHUNK = 8
VSHARD_PAD = CHUNK * NCHUNK     # 3840


# ----------------------------------------------------------------------------
# host-side constants
# ----------------------------------------------------------------------------

def _build_perm():
    """slot (0..1023) -> heap node index; slot 1023 is the pad."""
    perm = np.zeros(1024, dtype=np.int64)
    perm[0:512] = np.arange(511, 1023)      # blocks 0..3: leaves (level 9)
    perm[512:768] = np.arange(255, 511)     # blocks 4,5: level 8
    perm[768:896] = np.arange(127, 255)     # block 6: level 7
    p = 896                                 # block 7: levels 6..0 packed
    for lvl in range(6, -1, -1):
        s, e = 2 ** lvl - 1, 2 ** (lvl + 1) - 1
        perm[p:p + (e - s)] = np.arange(s, e)
        p += e - s
    perm[1023] = 0                          # pad slot gathers node-0's token (unused)
    return perm


def _build_mats(perm):
    SA = np.zeros((128, 128), np.float32)
    SB = np.zeros((128, 128), np.float32)
    for j in range(64):
        SA[2 * j, j] = 1.0
        SA[2 * j + 1, j] = 1.0
        SB[2 * j, j + 64] = 1.0
        SB[2 * j + 1, j + 64] = 1.0
    I = np.eye(128, dtype=np.float32)

    n7 = perm[896:1024]

    def is_anc(a, d):
        while True:
            if d == a:
                return True
            if d == 0:
                return False
            d = (d - 1) // 2

    A7 = np.zeros((128, 128), np.float32)
    B7 = np.zeros((128, 128), np.float32)
    w7 = np.zeros((128,), np.float32)
    for iloc in range(127):
        i = int(n7[iloc])
        for jloc in range(127):
            if is_anc(i, int(n7[jloc])):
                A7[jloc, iloc] = 1.0
        for k in range(128):
            if is_anc(i, 127 + k):
                B7[k, iloc] = 1.0
        lvl = int(np.floor(np.log2(i + 1)))
        w7[iloc] = 2.0 ** (8 - lvl) - 2.0
    # pad slot duplicates slot 0 so it can never incorrectly win the max
    A7[:, 127] = A7[:, 0]
    B7[:, 127] = B7[:, 0]
    w7[127] = w7[0]
    w2 = np.full((128,), 2.0, np.float32)
    return SA, SB, I, A7, B7, w2, w7


def _host_constants(W_c, b_c):
    perm = _build_perm()
    SA, SB, I, A7, B7, w2, w7 = _build_mats(perm)
    ident = np.eye(128, dtype=np.float32)
    mats = np.zeros((128, 7, 128), np.float32)
    for j, m in enumerate([SA, SB, I, A7, B7, ident]):
        mats[:, j, :] = m
    mats[:, 6, 0] = b_c
    brow = np.zeros((1, 768), np.float32)
    brow[0, 0:128] = w2
    brow[0, 128:256] = w7
    brow[0, 256:768] = np.tile(b_c, G)
    return perm, mats, brow


def _host_idx(tokens_core, perm):
    """tokens_core [32, 1023] -> idxw [128, 8, 4, 64] int16 (one 1024-idx gather
    per tree, wrapped in 16 partitions, replicated to 128)."""
    tok_pad = np.concatenate(
        [tokens_core, tokens_core[:, 0:1]], axis=1)       # [32, 1024]; pad col unused
    slots = tok_pad[:, perm]                              # [32, 1024], slot = B*128+p
    idxw = np.zeros((NGROUP, G, 16, 64), np.int16)
    for g in range(NGROUP):
        for t in range(G):
            idxw[g, t] = slots[g * G + t].reshape(64, 16).T
    idxw = np.tile(idxw, (1, 1, 8, 1))                    # [8, 4, 128, 64]
    return np.ascontiguousarray(idxw.transpose(2, 0, 1, 3))  # [128, 8, 4, 64]


# ----------------------------------------------------------------------------
# pass 1: table = emb @ W_c.T, vocab-sharded; output transposed [128, VSHARD_PAD]
# ----------------------------------------------------------------------------

def build_tablegen():
    nc = bacc.Bacc("TRN2", target_bir_lowering=False, debug=False)
    embt = nc.dram_tensor("embt", [128, VSHARD_PAD], F32R, kind="ExternalInput")
    wct = nc.dram_tensor("wct", [128, 128], F32R, kind="ExternalInput")
    tsh = nc.dram_tensor("tsh", [128, VSHARD_PAD], F32R, kind="ExternalOutput")
    with ExitStack() as ctx:
        embt_s = ctx.enter_context(nc.sbuf_tensor("embt_s", [128, VSHARD_PAD], F32R))
        wct_s = ctx.enter_context(nc.sbuf_tensor("wct_s", [128, 128], F32R))
        out_s = ctx.enter_context(nc.sbuf_tensor("out_s", [128, VSHARD_PAD], F32R))
        ps = [ctx.enter_context(nc.psum_tensor(f"ps{i}", [128, CHUNK], F32))
              for i in range(2)]
        s_in = ctx.enter_context(nc.semaphore("s_in"))
        s_mm = ctx.enter_context(nc.semaphore("s_mm"))
        s_cp = ctx.enter_context(nc.semaphore("s_cp"))
        s_out = ctx.enter_context(nc.semaphore("s_out"))
        with nc.Block() as block:
            @block.sync
            def _(eng):
                eng.dma_start(out=embt_s[:, :], in_=embt[:, :]).then_inc(s_in, 16)
                eng.dma_start(out=wct_s[:, :], in_=wct[:, :]).then_inc(s_in, 16)
                eng.wait_ge(s_cp, NCHUNK)
                eng.dma_start(out=tsh[:, :], in_=out_s[:, :]).then_inc(s_out, 16)
                eng.wait_ge(s_out, 16)

            @block.tensor
            def _(eng):
                eng.wait_ge(s_in, 32)
                for k in range(NCHUNK):
                    if k >= 2:
                        eng.wait_ge(s_cp, k - 1)
                    eng.matmul(
                        ps[k % 2][:, :],
                        wct_s[:, :],
                        embt_s[:, k * CHUNK:(k + 1) * CHUNK],
                        start=True, stop=True,
                    ).then_inc(s_mm, 1)

            @block.scalar
            def _(eng):
                for k in range(NCHUNK):
                    eng.wait_ge(s_mm, k + 1)
                    eng.activation(
                        out_s[:, k * CHUNK:(k + 1) * CHUNK], ps[k % 2][:, :],
                        AF.Copy,
                    ).then_inc(s_cp, 1)
    nc.compile()
    return nc


# ----------------------------------------------------------------------------
# pass 2: gather + tree-sum matmuls + max
# ----------------------------------------------------------------------------

def build_treeenc():
    nc = bacc.Bacc("TRN2", target_bir_lowering=False, debug=False,
                   num_swdge_queues=4)
    table = nc.dram_tensor("table", [VOCAB, ENC], F32R, kind="ExternalInput")
    idxw = nc.dram_tensor("idxw", [128, NGROUP, 4, 64], I16, kind="ExternalInput")
    matsd = nc.dram_tensor("mats", [128, 7, 128], F32R, kind="ExternalInput")
    browd = nc.dram_tensor("brow", [1, 768], F32R, kind="ExternalInput")
    m_out = nc.dram_tensor("m_out", [128, TPC], F32, kind="ExternalOutput")

    with ExitStack() as ctx:
        c_b = [ctx.enter_context(nc.sbuf_tensor(f"c{q}", [128, 4, 8, 128], F32R))
               for q in range(2)]
        h_b = [ctx.enter_context(nc.sbuf_tensor(f"h{q}", [128, 4, 512], F32R))
               for q in range(2)]
        mx_b = [ctx.enter_context(nc.sbuf_tensor(f"mx{q}", [128, 512], F32))
                for q in range(2)]
        tmpL = ctx.enter_context(nc.sbuf_tensor("tmpL", [128, 4, 2, 128], F32))
        tmp = [ctx.enter_context(nc.sbuf_tensor(f"tmp{i}", [128, 512], F32))
               for i in range(4)]
        M2_s = ctx.enter_context(nc.sbuf_tensor("M2_s", [128, TPC], F32))
        idx_s = ctx.enter_context(nc.sbuf_tensor("idx_s", [128, NGROUP, 4, 64], I16))
        mats_s = ctx.enter_context(nc.sbuf_tensor("mats_s", [128, 7, 128], F32R))
        brow_s = ctx.enter_context(nc.sbuf_tensor("brow_s", [1, 768], F32R))
        M_s = ctx.enter_context(nc.sbuf_tensor("M_s", [128, TPC], F32))

        # PSUM: 4 banks per parity: A, B, C, D.  A doubles as transpose target.
        pa = [ctx.enter_context(nc.psum_tensor(f"pa{q}", [128, 4, 128], F32))
              for q in range(2)]
        pb = [ctx.enter_context(nc.psum_tensor(f"pb{q}", [128, 4, 128], F32))
              for q in range(2)]
        pc = [ctx.enter_context(nc.psum_tensor(f"pc{q}", [128, 4, 128], F32))
              for q in range(2)]
        pd = [ctx.enter_context(nc.psum_tensor(f"pd{q}", [128, 4, 128], F32))
              for q in range(2)]

        s_init = ctx.enter_context(nc.semaphore("s_init"))
        s_gat = [[ctx.enter_context(nc.semaphore(f"s_gat{q}_{t}"))
                  for t in range(G)] for q in range(2)]
        s_mm = [ctx.enter_context(nc.semaphore(f"s_mm{x}")) for x in "ABCD"]
        s_act = [ctx.enter_context(nc.semaphore(f"s_act{x}")) for x in "ABCD"]
        s_leaf = ctx.enter_context(nc.semaphore("s_leaf"))
        s_mx = ctx.enter_context(nc.semaphore("s_mx"))
        s_tr = ctx.enter_context(nc.semaphore("s_tr"))
        s_red = ctx.enter_context(nc.semaphore("s_red"))
        s_fin = ctx.enter_context(nc.semaphore("s_fin"))
        s_out = ctx.enter_context(nc.semaphore("s_out"))
        s_gfin = ctx.enter_context(nc.semaphore("s_gfin"))

        SAa = mats_s[:, 0, :]
        SBa = mats_s[:, 1, :]
        Ia = mats_s[:, 2, :]
        A7a = mats_s[:, 3, :]
        B7a = mats_s[:, 4, :]
        identa = mats_s[:, 5, :].bitcast(F32)
        bcol = mats_s[:, 6, 0:1].bitcast(F32)
        w2a = brow_s[:, 0:128]
        w7a = brow_s[:, 128:256]
        bcra = brow_s[:, 256:768]

        def cblk(q, B):
            return c_b[q][:, :, B, :]

        with nc.Block(no_gpsimd_drain=True) as block:
            @block.sync
            def _(eng):
                eng.dma_start(out=idx_s[:, :, :, :], in_=idxw[:, :, :, :]).then_inc(s_init, 16)
                eng.dma_start(out=mats_s[:, :, :], in_=matsd[:, :, :]).then_inc(s_init, 16)
                eng.dma_start(out=brow_s[:, :], in_=browd[:, :]).then_inc(s_init, 16)
                eng.wait_ge(s_fin, 1)
                eng.dma_start(out=m_out[:, :], in_=M2_s[:, :]).then_inc(s_out, 16)
                eng.wait_ge(s_out, 16)

            @block.gpsimd
            def _(eng):
                eng.load_library(mlp)
                eng.wait_ge(s_init, 48)
                for g in range(NGROUP):
                    q = g % 2
                    if g >= 2:
                        eng.wait_ge(s_mm[3], g - 1)   # PE done with c[q] (bankD reads c7)
                        eng.wait_ge(s_leaf, g - 1)    # DVE done with leaf blocks of c[q]
                    for t in range(G):
                        eng.dma_gather(
                            c_b[q][:, t, :, :], table[:, :], idx_s[:, g, t, :],
                            1024, 1024, ENC, queue_num=t,
                        ).then_inc(s_gat[q][t], 16)

            @block.tensor
            def _(eng):
                eng.wait_ge(s_init, 48)
                for g in range(NGROUP):
                    q = g % 2
                    for t in range(G):
                        eng.wait_ge(s_gat[q][t], 16 * (g // 2 + 1))
                    # bank A: level-8 parents (tree-block 4)
                    if g >= 2:
                        eng.wait_ge(s_red, g - 1)     # pa[q] free (DVE reduce done)
                    eng.matmul(pa[q][:, :, :], SAa, cblk(q, 0), start=True, stop=False)
                    eng.matmul(pa[q][:, :, :], SBa, cblk(q, 1), start=False, stop=False)
                    eng.matmul(pa[q][:, :, :], Ia, cblk(q, 4), start=False, stop=False)
                    eng.matmul(pa[q][:, :, :], w2a, bcra, start=False, stop=True
                               ).then_inc(s_mm[0], 1)
                    # bank B: level-8 parents (tree-block 5)
                    if g >= 2:
                        eng.wait_ge(s_act[1], g - 1)  # pb[q] free
                    eng.matmul(pb[q][:, :, :], SAa, cblk(q, 2), start=True, stop=False)
                    eng.matmul(pb[q][:, :, :], SBa, cblk(q, 3), start=False, stop=False)
                    eng.matmul(pb[q][:, :, :], Ia, cblk(q, 5), start=False, stop=False)
                    eng.matmul(pb[q][:, :, :], w2a, bcra, start=False, stop=True
                               ).then_inc(s_mm[1], 1)
                    # bank C: level-7 parents, children are h0, h1
                    eng.wait_ge(s_act[0], g + 1)
                    eng.wait_ge(s_act[1], g + 1)
                    if g >= 2:
                        eng.wait_ge(s_act[2], g - 1)  # pc[q] free
                    eng.matmul(pc[q][:, :, :], SAa, h_b[q][:, 0, :],
                               start=True, stop=False)
                    eng.matmul(pc[q][:, :, :], SBa, h_b[q][:, 1, :],
                               start=False, stop=False)
                    eng.matmul(pc[q][:, :, :], Ia, cblk(q, 6), start=False, stop=False)
                    eng.matmul(pc[q][:, :, :], w2a, bcra, start=False, stop=True
                               ).then_inc(s_mm[2], 1)
                    # bank D: packed top block (levels 6..0)
                    eng.wait_ge(s_act[2], g + 1)
                    if g >= 2:
                        eng.wait_ge(s_act[3], g - 1)  # pd[q] free
                    eng.matmul(pd[q][:, :, :], A7a, cblk(q, 7), start=True, stop=False)
                    eng.matmul(pd[q][:, :, :], B7a, h_b[q][:, 2, :],
                               start=False, stop=False)
                    eng.matmul(pd[q][:, :, :], w7a, bcra, start=False, stop=True
                               ).then_inc(s_mm[3], 1)
                    # transposes of the block-max into pa[q] (pa already copied out)
                    eng.wait_ge(s_mx, g + 1)
                    for t in range(G):
                        ins = eng.transpose(pa[q][:, t, :],
                                            mx_b[q][:, t * 128:(t + 1) * 128],
                                            identa)
                    ins.then_inc(s_tr, 1)

            @block.scalar
            def _(eng):
                for g in range(NGROUP):
                    q = g % 2
                    if g >= 2:
                        eng.wait_ge(s_mx, g - 1)      # h[q] free (DVE maxes done)
                    for x in range(4):
                        eng.wait_ge(s_mm[x], g + 1)
                        src = [pa, pb, pc, pd][x]
                        eng.activation(h_b[q][:, x, :], src[q][:, :, :], AF.Copy
                                       ).then_inc(s_act[x], 1)

            @block.vector
            def _(eng):
                for g in range(NGROUP):
                    q = g % 2
                    for t in range(G):
                        eng.wait_ge(s_gat[q][t], 16 * (g // 2 + 1))
                    # leaf max over c blocks 0..3 (per tree)
                    eng.tensor_max(tmpL[:, :, :, :], c_b[q][:, :, 0:2, :].bitcast(F32),
                                   c_b[q][:, :, 2:4, :].bitcast(F32)
                                   ).then_inc(s_leaf, 1)
                    # internal max over h blocks
                    eng.wait_ge(s_act[3], g + 1)
                    eng.tensor_max(tmp[0][:, :], h_b[q][:, 0, :].bitcast(F32),
                                   h_b[q][:, 1, :].bitcast(F32))
                    eng.tensor_max(tmp[1][:, :], h_b[q][:, 2, :].bitcast(F32),
                                   h_b[q][:, 3, :].bitcast(F32))
                    eng.drain()
                    eng.tensor_max(tmp[2][:, :], tmpL[:, :, 0, :], tmpL[:, :, 1, :])
                    eng.tensor_max(tmp[3][:, :], tmp[0][:, :], tmp[1][:, :])
                    eng.drain()
                    if g >= 2:
                        eng.wait_ge(s_tr, g - 1)      # mx[q] free (PE transposes done)
                    eng.tensor_max(mx_b[q][:, :], tmp[2][:, :], tmp[3][:, :]
                                   ).then_inc(s_mx, 1)
                    # per-tree reduction of transposed maxes
                    eng.wait_ge(s_tr, g + 1)
                    eng.reduce_max(M_s[:, G * g:G * (g + 1)], pa[q][:, :, :],
                                   axis=mybir.AxisListType.X).then_inc(s_red, 1)
                eng.drain()
                eng.tensor_scalar_add(M2_s[:, :], M_s[:, :], bcol).then_inc(s_fin, 1)
    nc.compile()
    return nc


# ----------------------------------------------------------------------------
# host orchestration
# ----------------------------------------------------------------------------

_CACHE = {}


def _get_programs():
    if "nc1" not in _CACHE:
        _CACHE["nc1"] = build_tablegen()
        _CACHE["nc2"] = build_treeenc()
    return _CACHE["nc1"], _CACHE["nc2"]


def _pass1_inputs(emb, W_c):
    embT = np.zeros((128, NCORES, VSHARD_PAD), np.float32)
    embT[:, :, :VSHARD] = np.ascontiguousarray(emb.T).reshape(128, NCORES, VSHARD)
    wct = np.ascontiguousarray(W_c.T)
    return [{"embt": np.ascontiguousarray(embT[:, i, :]), "wct": wct}
            for i in range(NCORES)]


def _assemble_table(results):
    shards = [results[i]["tsh"][:, :VSHARD] for i in range(NCORES)]
    return np.ascontiguousarray(np.concatenate(shards, axis=1).T)  # [30000, 128]


def _pass2_inputs(tokens, table, W_c, b_c):
    perm, mats, brow = _host_constants(W_c, b_c)
    in2 = []
    for i in range(NCORES):
        tc = tokens[i * TPC:(i + 1) * TPC]
        in2.append({
            "table": table,
            "idxw": _host_idx(tc, perm),
            "mats": mats,
            "brow": brow,
        })
    return in2


def _assemble_out(results):
    out = np.zeros((BS, ENC), np.float32)
    for i in range(NCORES):
        out[i * TPC:(i + 1) * TPC] = results[i]["m_out"].T
    return out


def kernel(tokens, emb, W_c, b_c, bs=None):
    tokens = np.asarray(tokens).astype(np.int64)
    emb = np.asarray(emb, np.float32)
    W_c = np.asarray(W_c, np.float32)
    b_c = np.asarray(b_c, np.float32)
    assert tokens.shape == (BS, NNODE) and emb.shape == (VOCAB, EMB)

    nc1, nc2 = _get_programs()
    core_ids = list(range(NCORES))

    r1 = run_bass_kernel_spmd(nc1, _pass1_inputs(emb, W_c), core_ids)
    table = _assemble_table(r1.results)
    r2 = run_bass_kernel_spmd(nc2, _pass2_inputs(tokens, table, W_c, b_c),
                              core_ids)
    return _assemble_out(r2.results)
